# revision 51
# baseline (speedup 1.0000x reference)
"""Multi-head attention (B=2, L=S=2048, D=1024, H=16, E=64) on 8 TRN2 cores.

Sharding: tensor-parallel over heads. Core c owns heads 2c, 2c+1, i.e. the
128-wide slice [c*128:(c+1)*128] of the QKV projection outputs and the
matching row-slice of Wo. Each core reads the full (host-pre-arranged)
queries/keys/values, computes its two heads' attention, and writes a full
[1024, 4096] bf16 partial of the output projection; the host sums the 8
partials, transposes back and adds bo.

Per-core structure (all bf16 matmuls, f32 PSUM):
  proj     q/k/v^T = W^T X^T -> [128 e2, 512] bf16 per token-tile
  scores^T sc[128 s, (2h x 512 l)] f32 PSUM per s-tile
  P^T      pt = exp(sc/8) [128 s, 1024] bf16 (ScalarE, the critical engine)
  PV       out[128 l, 65] per (h, l128): lhsT=pt-slice, rhs=[V_h | 1],
           accumulated over 16 s-tiles; col 64 = softmax denominator.
           PV matmuls are emitted 5 slots behind their exp so the PE queue
           head never waits on the Activation engine.
  norm     per-partition reciprocal + tensor_scalar (DVE/Pool split)
  A^T      SP DMA-transpose [128 l, 128 e] -> OT [128 e2, L]
  outproj  Wo_c^T @ OT -> [128 d, 512] -> bf16 -> one 8KB/partition DMA/unit
Host: sums the 8 bf16 partials in fp64, un-permutes, adds bo.
"""

from collections import deque

import numpy as np
import ml_dtypes

import concourse.bass as bass
import concourse.bacc as bacc
import concourse.mybir as mybir
from concourse.tile import TileContext
from concourse.bass_utils import run_bass_kernel_spmd

BF16 = mybir.dt.bfloat16
F32 = mybir.dt.float32

B, L, D = 2, 2048, 1024
TOK = B * L              # 4096
H, E = 16, 64
NCORES = 8
E2 = 128                 # projection output dims per core (2 heads)
NKT = D // 128           # 8 k-tiles of the contraction
LQ = 512                 # l-quarter: query-token tile inside attention
NLQ = L // LQ            # 4 per batch
NST = L // 128           # 16 s-tiles (key tokens) per batch
HEADS = 2                # heads per core
PV_LAG = 5               # slots between exp(st) and its PV matmuls

_CACHED_NC = None
_IDENT = np.eye(128, dtype=ml_dtypes.bfloat16)


def _warrange(w):
    # [D, E2] -> [128, NKT*E2]: row p holds [w[kt*128+p, :] for kt]
    return np.ascontiguousarray(
        w.reshape(NKT, 128, E2).transpose(1, 0, 2).reshape(128, NKT * E2)
    ).astype(ml_dtypes.bfloat16)


def build_nc():
    nc = bacc.Bacc("TRN2", target_bir_lowering=False)

    xp = {n: nc.declare_dram_parameter(f"x{n}", [128, NKT, TOK], BF16,
                                       isOutput=False)
          for n in ("q", "k", "v")}
    w = {n: nc.declare_dram_parameter(f"w{n}", [128, NKT * E2], BF16,
                                      isOutput=False)
         for n in ("q", "k", "v")}
    bias = {n: nc.declare_dram_parameter(f"b{n}", [E2, 1], F32, isOutput=False)
            for n in ("q", "k", "v")}
    wo = nc.declare_dram_parameter("wo", [E2, D], BF16, isOutput=False)
    ident_in = nc.declare_dram_parameter("ident_in", [128, 128], BF16,
                                         isOutput=False)
    out_t = nc.declare_dram_parameter("out_t", [128, 8 * 8 * 512], BF16,
                                      isOutput=True)

    with TileContext(nc) as tc:
        with (
            tc.tile_pool(name="const", bufs=1) as const,
            tc.tile_pool(name="persist", bufs=1) as persist,
            tc.tile_pool(name="xt_pool", bufs=9) as xt_pool,
            tc.tile_pool(name="pt_pool", bufs=8) as pt_pool,
            tc.tile_pool(name="a_pool", bufs=6) as a_pool,
            tc.tile_pool(name="rec_pool", bufs=8) as rec_pool,
            tc.tile_pool(name="pvc_pool", bufs=2) as pvc_pool,
            tc.tile_pool(name="ob_pool", bufs=3) as ob_pool,
            tc.tile_pool(name="sc_ps", bufs=2, space="PSUM") as sc_ps,
            tc.tile_pool(name="misc_ps", bufs=2, space="PSUM") as misc_ps,
            tc.tile_pool(name="pv_ps", bufs=1, space="PSUM") as pv_ps,
        ):
            # ---- constants. w_k/w_q ride the SP (HWDGE) queue interleaved
            # with the first x chunks (ordering on the serial DMA device);
            # the small ones (biases, ident) and w_v/wo go via SWDGE so
            # they don't add HWDGE issue latency to the critical path ----
            w_sb = {n: const.tile([128, NKT * E2], BF16, tag=f"w_{n}",
                                  name=f"w_{n}") for n in ("k", "v", "q")}
            b_sb, b_dmas = {}, {}
            for n in ("k", "v", "q"):
                b_dmas[n] = const.tile([E2, 1], F32, tag=f"bdma_{n}",
                                       name=f"bdma_{n}")
                b_sb[n] = const.tile([E2, 1], F32, tag=f"b_{n}", name=f"b_{n}")
            ident = const.tile([128, 128], BF16, tag="ident")
            wo_sb = const.tile([E2, D], BF16, tag="wo")
            warm = const.tile([1, 2], F32, tag="warm")

            def const_loads_first():
                # issue pipeline is ~1.3us per DMA at the head of the
                # program, so only w_k may precede the first x chunk
                nc.vector.memset(warm[:], 0.0)
                nc.scalar.activation(warm[:], warm[:],
                                     mybir.ActivationFunctionType.Exp)
                nc.sync.dma_start(out=w_sb["k"][:], in_=w["k"].ap())

            def const_loads_early():
                for n in ("k", "v", "q"):
                    nc.sync.dma_start(out=b_dmas[n][:], in_=bias[n].ap())
                    nc.vector.tensor_copy(b_sb[n][:], b_dmas[n][:])
                nc.sync.dma_start(out=ident[:], in_=ident_in.ap())
                nc.sync.dma_start(out=w_sb["q"][:], in_=w["q"].ap())

            def const_loads_mid():
                nc.sync.dma_start(out=w_sb["v"][:], in_=w["v"].ap())

            def const_loads_late():
                nc.sync.dma_start(out=wo_sb[:], in_=wo.ap())

            # ---- persistent tensors ----
            qt_sbs = [[persist.tile([E2, LQ], BF16, tag=f"qt{b}_{t}",
                                    name=f"qt{b}_{t}") for t in range(4)]
                      for b in range(B)]
            kt_sbs = [[persist.tile([E2, LQ], BF16, tag=f"kt{b}_{t}",
                                    name=f"kt{b}_{t}") for t in range(4)]
                      for b in range(B)]
            vt_sbs = [[persist.tile([E2, LQ], BF16, tag=f"vt{b}_{t}",
                                    name=f"vt{b}_{t}") for t in range(4)]
                      for b in range(B)]
            # V' per head: [128 s-part, (4 r, 65)] with col 64 == 1.0
            vp_sbs = [[[persist.tile([128, 4 * 65], BF16, tag=f"vp{b}_{h}_{g}",
                                     name=f"vp{b}_{h}_{g}") for g in range(4)]
                       for h in range(HEADS)] for b in range(B)]
            ot_sbs = [persist.tile([E2, L], BF16, tag=f"ot{b}", name=f"ot{b}")
                      for b in range(B)]

            for b in range(B):
                for h in range(HEADS):
                    for g in range(4):
                        nc.gpsimd.memset(vp_sbs[b][h][g][:], 1.0)

            proj_dst = {"q": qt_sbs, "k": kt_sbs, "v": vt_sbs}
            xchunks = {}
            accs = {}

            def dma_x(n, b, tt):
                t0 = b * L + tt * LQ
                xt = xt_pool.tile([128, NKT, LQ], BF16, tag="xt",
                                  name=f"x{n}{b}{tt}")
                nc.sync.dma_start(out=xt[:], in_=xp[n].ap()[:, :, t0:t0 + LQ])
                xchunks[(n, b, tt)] = xt

            def proj_chunk(n, b, tt, ci):
                """Two k-tile matmuls of the (n,b,tt) projection; ci in 0..3."""
                if ci == 0:
                    accs[(n, b, tt)] = misc_ps.tile([128, LQ], F32, tag="mps",
                                                    name=f"acc{n}{b}{tt}")
                acc = accs[(n, b, tt)]
                xt = xchunks[(n, b, tt)]
                for kt in (2 * ci, 2 * ci + 1):
                    nc.tensor.matmul(
                        acc[:],
                        lhsT=w_sb[n][:, kt * E2:(kt + 1) * E2],
                        rhs=xt[:, kt, :],
                        start=(kt == 0),
                        stop=(kt == NKT - 1),
                    )
                if ci == 3:
                    dst = proj_dst[n][b][tt]
                    nc.vector.tensor_scalar_add(dst[:], acc[:], b_sb[n][:])
                    del accs[(n, b, tt)], xchunks[(n, b, tt)]

            def proj_full(n, b, tt):
                for ci in range(4):
                    proj_chunk(n, b, tt, ci)

            def transp(b, g):
                """VT g-tile [128 e2, 512 s] -> vp [128 s, (r, 65)]."""
                for r in range(4):
                    tp = misc_ps.tile([128, 128], BF16, tag="mps",
                                      name=f"tp{b}{g}{r}")
                    nc.tensor.transpose(
                        tp[:], vt_sbs[b][g][:, r * 128:(r + 1) * 128], ident[:]
                    )
                    for h in range(HEADS):
                        nc.vector.tensor_copy(
                            vp_sbs[b][h][g][:, r * 65:r * 65 + 64],
                            tp[:, h * 64:(h + 1) * 64],
                        )

            def drain_copy(b, lq, pv):
                """bulk copy PSUM->SBUF: releases the single pv buffer fast
                so the next unit's memset+accumulation isn't stalled behind
                the full normalize chain."""
                pvc = pvc_pool.tile([128, 1024], F32, tag="pvc",
                                    name=f"pvc_{b}_{lq}")
                nc.vector.tensor_copy(pvc[:, 0:260], pv[:, 0:260])
                nc.vector.tensor_copy(pvc[:, 512:772], pv[:, 512:772])
                return pvc

            def drain_norm(b, lq, pvc):
                """normalize pvc -> A tiles -> transpose into OT.

                b=0 units transpose via the SP DMA XBAR (PE and the shared
                mps PSUM slots are busy with projection accumulators then);
                b=1 units use PE transposes -- ~1.3us SP issue latency per
                DMA-transpose otherwise delays ot and stalls the next
                unit's out-projection at the PE queue head."""
                l0 = lq * LQ
                As = []
                for lt in range(4):
                    A = a_pool.tile([128, 128], BF16, tag="asb",
                                    name=f"a_{b}_{lq}_{lt}")
                    for h in range(HEADS):
                        gi = h * 4 + lt
                        rec = rec_pool.tile([128, 1], F32, tag="rec",
                                            name=f"rec_{b}_{lq}_{gi}")
                        nc.vector.reciprocal(
                            rec[:], pvc[:, gcol(gi) + 64:gcol(gi) + 65]
                        )
                        eng = nc.vector if h == 0 else nc.gpsimd
                        eng.tensor_scalar_mul(
                            A[:, h * 64:(h + 1) * 64],
                            pvc[:, gcol(gi):gcol(gi) + 64],
                            rec[:],
                        )
                    As.append((lt, A))

                def emit_transposes(b=b, l0=l0, As=tuple(As)):
                    for lt, A in As:
                        tp = misc_ps.tile([128, 128], BF16, tag="mps",
                                          name=f"at{b}{l0}{lt}")
                        nc.tensor.transpose(tp[:], A[:], ident[:])
                        nc.vector.tensor_copy(
                            ot_sbs[b][:, l0 + lt * 128:l0 + (lt + 1) * 128],
                            tp[:],
                        )
                if b == 0:
                    # defer to the st4 pop point: the only moment when no
                    # projection accumulator holds the shared mps PSUM slots
                    transp_pending.append(emit_transposes)
                else:
                    emit_transposes()

            obs = {}
            copyq = deque()

            def outproj_dt(b, lq, dt, tail=False):
                if dt == 0:
                    obs[(b, lq)] = ob_pool.tile([128, 8 * LQ], BF16, tag="ob",
                                                name=f"ob_{b}_{lq}")
                l0 = lq * LQ
                if tail and dt % 2:
                    # the sc pool is idle at the tail: alternating pools
                    # gives 4 PSUM rotation slots so out-proj pairs don't
                    # serialize on the 2-buffer mps rotation
                    op = sc_ps.tile([128, LQ], F32, tag="sc",
                                    name=f"op{b}{lq}{dt}")
                else:
                    op = misc_ps.tile([128, LQ], F32, tag="mps",
                                      name=f"op{b}{lq}{dt}")
                nc.tensor.matmul(
                    op[:],
                    lhsT=wo_sb[:, dt * 128:(dt + 1) * 128],
                    rhs=ot_sbs[b][:, l0:l0 + LQ],
                    start=True, stop=True,
                )
                ob = obs[(b, lq)]

                def do_copy():
                    if tail and dt % 2:
                        nc.scalar.copy(ob[:, dt * LQ:(dt + 1) * LQ], op[:])
                    else:
                        nc.vector.tensor_copy(ob[:, dt * LQ:(dt + 1) * LQ],
                                              op[:])
                if tail:
                    do_copy()
                else:
                    # defer the DVE copy ~2 slots so it never parks at the
                    # DVE queue head while its matmul is still pending
                    copyq.append(do_copy)

            def outdma(b, lq):
                while copyq:  # all copies for this unit must precede its DMA
                    copyq.popleft()()
                u = b * NLQ + lq
                nc.sync.dma_start(
                    out=out_t.ap()[:, u * 4096:(u + 1) * 4096],
                    in_=obs.pop((b, lq))[:],
                )

            def dummy():
                # keeps the PE p-state ramped through Act-bound stretches
                dm = misc_ps.tile([128, LQ], F32, tag="mps", name="dummy")
                nc.tensor.matmul(dm[:], lhsT=ident[:], rhs=w_sb["k"][:, 0:LQ],
                                 start=True, stop=True)

            pv_tiles = {}
            norm_pending = []
            transp_pending = []

            def gcol(gi):
                # 4 x 65-wide groups per 512-f32 PSUM bank
                return (gi // 4) * 512 + (gi % 4) * 65

            def emit_pv(b, lq, st, pt):
                if st == 0:
                    pv_tiles[(b, lq)] = pv_ps.tile([128, 1024], F32,
                                                   tag="pv",
                                                   name=f"pv_{b}_{lq}")
                pv = pv_tiles[(b, lq)]
                g, r = st // 4, st % 4
                for h in range(HEADS):
                    for lt in range(4):
                        gi = h * 4 + lt
                        # start=True on the first group of each PSUM bank
                        # marks the whole bank pending-zero; the other
                        # groups accumulate onto lazily-zeroed bytes
                        nc.tensor.matmul(
                            pv[:, gcol(gi):gcol(gi) + 65],
                            lhsT=pt[:, h * LQ + lt * 128:
                                    h * LQ + (lt + 1) * 128],
                            rhs=vp_sbs[b][h][g][:, r * 65:(r + 1) * 65],
                            start=(st == 0 and gi % 4 == 0),
                            stop=(st == NST - 1),
                            skip_group_check=True,
                        )
                if st == 0 and norm_pending:
                    drain_norm(*norm_pending.pop(0))
                if st == 4 and transp_pending:
                    transp_pending.pop(0)()
                if st == NST - 1:
                    pvc = drain_copy(b, lq, pv_tiles.pop((b, lq)))
                    norm_pending.append((b, lq, pvc))

            def emit_sc_exp(b, lq, st):
                g, r = st // 4, st % 4
                sc = sc_ps.tile([128, 2 * LQ], F32, tag="sc",
                                name=f"sc_{b}_{lq}_{st}")
                for h in range(HEADS):
                    nc.tensor.matmul(
                        sc[:, h * LQ:(h + 1) * LQ],
                        lhsT=kt_sbs[b][g][h * 64:(h + 1) * 64,
                                          r * 128:(r + 1) * 128],
                        rhs=qt_sbs[b][lq][h * 64:(h + 1) * 64, :],
                        start=True, stop=True,
                    )
                pt = pt_pool.tile([128, 2 * LQ], BF16, tag="pt",
                                  name=f"pt_{b}_{lq}_{st}")
                nc.scalar.activation(
                    pt[:], sc[:], mybir.ActivationFunctionType.Exp,
                    scale=0.125,
                )
                return pt

            # ---------------- schedule ----------------
            def F(fn, *a):
                return lambda: fn(*a)

            def pc(n, b, tt, ci):
                return F(proj_chunk, n, b, tt, ci)

            def opj(b, lq, dt):
                return F(outproj_dt, b, lq, dt)

            # prologue: k00/q00 loads + projections seed unit (0,0); the
            # v00 projection rides unit 0's filler slots (PV lags 5 slots)
            const_loads_first()
            dma_x("k", 0, 0)
            dma_x("q", 0, 0)
            const_loads_early()
            dma_x("v", 0, 0)
            const_loads_mid()
            const_loads_late()
            for _ in range(6):  # burn the PE p-state ramp on w_k
                dm = misc_ps.tile([128, LQ], F32, tag="mps", name="dwarm")
                nc.tensor.matmul(dm[:], lhsT=w_sb["k"][:, 0:128],
                                 rhs=w_sb["k"][:, 0:LQ], start=True, stop=True)
            proj_full("k", 0, 0)
            proj_full("q", 0, 0)

            # per-unit DMA emissions (SP queue order ~ transfer order)
            unit_dmas = {
                0: [("k", 0, 1), ("v", 0, 1), ("q", 0, 1), ("k", 0, 2),
                    ("v", 0, 2), ("k", 0, 3), ("v", 0, 3)],
                1: [("q", 0, 2), ("k", 1, 0), ("v", 1, 0)],
                2: [("q", 0, 3), ("k", 1, 1), ("v", 1, 1)],
                3: [("q", 1, 0), ("k", 1, 2), ("v", 1, 2), ("q", 1, 1)],
                4: [("k", 1, 3), ("v", 1, 3), ("q", 1, 2)],
                5: [("q", 1, 3)],
            }

            # unit slot fillers (PE work injected after each s-tile).
            # Invariant: every kt/qt/vp producer is EMITTED before the
            # sc/pv instruction that reads it (program order = PE queue
            # order), with PV emission lagging its s-tile by PV_LAG slots.
            slots = {}
            slots[0] = {
                0: [pc("v", 0, 0, 0), pc("k", 0, 1, 0)],
                1: [pc("v", 0, 0, 1), pc("k", 0, 1, 1)],
                2: [pc("v", 0, 0, 2), pc("k", 0, 1, 2)],
                3: [pc("v", 0, 0, 3), pc("k", 0, 1, 3), F(transp, 0, 0)],
                4: [pc("v", 0, 1, 0), pc("k", 0, 2, 0)],
                5: [pc("v", 0, 1, 1), pc("k", 0, 2, 1)],
                6: [pc("v", 0, 1, 2), pc("k", 0, 2, 2)],
                7: [pc("v", 0, 1, 3), pc("k", 0, 2, 3), F(transp, 0, 1)],
                8: [pc("v", 0, 2, 0), pc("k", 0, 3, 0)],
                9: [pc("v", 0, 2, 1), pc("k", 0, 3, 1)],
                10: [pc("v", 0, 2, 2), pc("k", 0, 3, 2)],
                11: [pc("v", 0, 2, 3), pc("k", 0, 3, 3), F(transp, 0, 2)],
                12: [pc("v", 0, 3, 0), pc("q", 0, 1, 0)],
                13: [pc("v", 0, 3, 1), pc("q", 0, 1, 1)],
                14: [pc("v", 0, 3, 2), pc("q", 0, 1, 2)],
                15: [pc("v", 0, 3, 3), pc("q", 0, 1, 3), F(transp, 0, 3)],
            }
            slots[1] = {
                0: [pc("q", 0, 2, 0)], 1: [pc("q", 0, 2, 1)],
                2: [pc("q", 0, 2, 2)], 3: [pc("q", 0, 2, 3)],
                # drain(u0) lands after the pv pop at slot 4
                4: [pc("k", 1, 0, 0)], 5: [pc("k", 1, 0, 1)],
                6: [pc("k", 1, 0, 2)], 7: [pc("k", 1, 0, 3)],
                8: [pc("v", 1, 0, 0)], 9: [pc("v", 1, 0, 1)],
                10: [pc("v", 1, 0, 2)],
                11: [pc("v", 1, 0, 3), F(transp, 1, 0)],
                12: [opj(0, 0, 0), opj(0, 0, 1)],
                13: [opj(0, 0, 2), opj(0, 0, 3)],
                14: [opj(0, 0, 4), opj(0, 0, 5)],
                15: [opj(0, 0, 6), opj(0, 0, 7)],
            }
            slots[2] = {
                0: [pc("q", 0, 3, 0)], 1: [pc("q", 0, 3, 1)],
                2: [pc("q", 0, 3, 2)], 3: [pc("q", 0, 3, 3)],
                4: [pc("k", 1, 1, 0)], 5: [pc("k", 1, 1, 1)],
                6: [pc("k", 1, 1, 2)], 7: [pc("k", 1, 1, 3)],
                8: [pc("v", 1, 1, 0)], 9: [pc("v", 1, 1, 1)],
                10: [pc("v", 1, 1, 2)],
                11: [pc("v", 1, 1, 3), F(transp, 1, 1)],
                12: [opj(0, 1, 0), opj(0, 1, 1)],
                13: [opj(0, 1, 2), opj(0, 1, 3)],
                14: [opj(0, 1, 4), opj(0, 1, 5)],
                15: [opj(0, 1, 6), opj(0, 1, 7)],
            }
            slots[3] = {
                0: [pc("q", 1, 0, 0)], 1: [pc("q", 1, 0, 1)],
                2: [pc("q", 1, 0, 2)], 3: [pc("q", 1, 0, 3)],
                4: [pc("k", 1, 2, 0)], 5: [pc("k", 1, 2, 1)],
                6: [pc("k", 1, 2, 2)], 7: [pc("k", 1, 2, 3)],
                8: [pc("v", 1, 2, 0)], 9: [pc("v", 1, 2, 1)],
                10: [pc("v", 1, 2, 2)],
                11: [pc("v", 1, 2, 3), F(transp, 1, 2)],
                12: [pc("q", 1, 1, 0)], 13: [pc("q", 1, 1, 1)],
                14: [pc("q", 1, 1, 2)], 15: [pc("q", 1, 1, 3)],
            }
            slots[4] = {
                0: [pc("k", 1, 3, 0)], 1: [pc("k", 1, 3, 1)],
                2: [pc("k", 1, 3, 2), F(outdma, 0, 0)],
                3: [pc("k", 1, 3, 3)],
                4: [pc("v", 1, 3, 0)], 5: [pc("v", 1, 3, 1)],
                6: [pc("v", 1, 3, 2)],
                7: [pc("v", 1, 3, 3), F(transp, 1, 3)],
                8: [opj(0, 2, 0), opj(0, 2, 1)],
                9: [opj(0, 2, 2), opj(0, 2, 3)],
                10: [opj(0, 2, 4), opj(0, 2, 5), F(outdma, 0, 1)],
                11: [opj(0, 2, 6), opj(0, 2, 7)],
                12: [pc("q", 1, 2, 0)], 13: [pc("q", 1, 2, 1)],
                14: [pc("q", 1, 2, 2)], 15: [pc("q", 1, 2, 3)],
            }
            slots[5] = {
                0: [opj(0, 3, 0), F(dummy)],
                1: [opj(0, 3, 1), F(dummy)],
                2: [opj(0, 3, 2), F(outdma, 0, 2), F(dummy)],
                3: [opj(0, 3, 3), F(dummy)],
                4: [opj(0, 3, 4), F(dummy)],
                5: [opj(0, 3, 5), F(dummy)],
                6: [opj(0, 3, 6), F(dummy)],
                7: [opj(0, 3, 7), F(dummy)],
                8: [pc("q", 1, 3, 0)],
                9: [pc("q", 1, 3, 1), F(outdma, 0, 3)],
                10: [pc("q", 1, 3, 2)], 11: [pc("q", 1, 3, 3)],
                12: [opj(1, 0, 0), F(dummy)],
                13: [opj(1, 0, 1), F(dummy)],
                14: [opj(1, 0, 2), F(dummy)],
                15: [opj(1, 0, 3), F(dummy)],
            }
            slots[6] = {
                0: [opj(1, 0, 4), F(dummy)],
                1: [opj(1, 0, 5), F(dummy)],
                2: [opj(1, 0, 6), F(dummy)],
                3: [opj(1, 0, 7), F(dummy)],
                4: [F(dummy), F(dummy)],
                5: [F(dummy), F(outdma, 1, 0), F(dummy)],
                6: [F(dummy), F(dummy)],
                7: [opj(1, 1, 0), F(dummy)],
                8: [opj(1, 1, 1), F(dummy)],
                9: [opj(1, 1, 2), F(dummy)],
                10: [opj(1, 1, 3), F(dummy)],
                11: [opj(1, 1, 4), F(dummy)],
                12: [opj(1, 1, 5), F(dummy)],
                13: [opj(1, 1, 6), F(dummy)],
                14: [opj(1, 1, 7), F(dummy)],
                15: [F(dummy), F(dummy)],
            }
            slots[7] = {
                0: [F(dummy), F(dummy)],
                1: [F(dummy), F(outdma, 1, 1), F(dummy)],
                2: [F(dummy), F(dummy)],
                3: [F(dummy), F(dummy)],
                4: [F(dummy), F(dummy)],
                5: [F(dummy), F(dummy)],
                6: [F(dummy), F(dummy)],
                7: [opj(1, 2, 0), F(dummy)],
                8: [opj(1, 2, 1), F(dummy)],
                9: [opj(1, 2, 2), F(dummy)],
                10: [opj(1, 2, 3), F(dummy)],
                11: [opj(1, 2, 4), F(dummy)],
                12: [opj(1, 2, 5), F(dummy)],
                13: [opj(1, 2, 6), F(dummy)],
                14: [opj(1, 2, 7), F(dummy)],
                15: [F(dummy), F(dummy)],
            }

            units = [(b, lq) for b in range(B) for lq in range(NLQ)]
            pending = deque()
            for u, (b, lq) in enumerate(units):
                for args in unit_dmas.get(u, []):
                    dma_x(*args)
                for st in range(NST):
                    pt = emit_sc_exp(b, lq, st)
                    pending.append((b, lq, st, pt))
                    if len(pending) > PV_LAG:
                        emit_pv(*pending.popleft())
                        # pull a unit's last PV batch (and its drain_copy)
                        # one slot early: the next unit's first PV then has
                        # two slots of cover for the pv-buffer WAR
                        if pending and pending[0][2] == NST - 1:
                            emit_pv(*pending.popleft())
                    for f in slots.get(u, {}).get(st, []):
                        f()
                    if len(copyq) > 2:
                        copyq.popleft()()
            while pending:
                emit_pv(*pending.popleft())
            while norm_pending:
                drain_norm(*norm_pending.pop(0))
            while transp_pending:
                transp_pending.pop(0)()
            while copyq:
                copyq.popleft()()
            outdma(1, 2)
            u = 7
            for pair in range(4):
                for dt in (2 * pair, 2 * pair + 1):
                    outproj_dt(1, 3, dt, tail=True)
                # quarter DMAs: the final transfer starts right after the
                # last copy instead of waiting behind a 2048-col half
                nc.sync.dma_start(
                    out=out_t.ap()[:, u * 4096 + pair * 2 * LQ:
                                   u * 4096 + (pair + 1) * 2 * LQ],
                    in_=obs[(1, 3)][:, pair * 2 * LQ:(pair + 1) * 2 * LQ],
                )
            obs.pop((1, 3))

    nc.compile()
    return nc


def _get_nc():
    global _CACHED_NC
    if _CACHED_NC is None:
        _CACHED_NC = build_nc()
    return _CACHED_NC


def _prep_inputs(queries, keys, values, Wq, bq, Wk, bk, Wv, bv, Wo, bo):
    bf16 = ml_dtypes.bfloat16
    x3 = {}
    for n, arr in (("q", queries), ("k", keys), ("v", values)):
        xt = np.asarray(arr, np.float32).reshape(TOK, D).T  # [D, TOK]
        x3[n] = np.ascontiguousarray(
            xt.reshape(NKT, 128, TOK).transpose(1, 0, 2)
        ).astype(bf16)  # [128, NKT, TOK]
    in_maps = []
    for c in range(NCORES):
        sl = slice(c * E2, (c + 1) * E2)
        m = {
            "xq": x3["q"], "xk": x3["k"], "xv": x3["v"],
            "wq": _warrange(np.asarray(Wq, np.float32)[:, sl]),
            "wk": _warrange(np.asarray(Wk, np.float32)[:, sl]),
            "wv": _warrange(np.asarray(Wv, np.float32)[:, sl]),
            "bq": np.ascontiguousarray(
                np.asarray(bq, np.float32)[sl].reshape(E2, 1)),
            "bk": np.ascontiguousarray(
                np.asarray(bk, np.float32)[sl].reshape(E2, 1)),
            "bv": np.ascontiguousarray(
                np.asarray(bv, np.float32)[sl].reshape(E2, 1)),
            "wo": np.ascontiguousarray(
                np.asarray(Wo, np.float32)[sl, :]).astype(bf16),
            "ident_in": _IDENT,
        }
        in_maps.append(m)
    return in_maps


def _postprocess(results, bo):
    acc = np.zeros((128, 8 * 8 * 512), np.float64)
    for r in results:
        acc += r["out_t"].astype(np.float64)  # bf16 partials, summed in fp64
    arr = acc.reshape(128, 8, 8, 512)         # [p, unit, dt, t]
    full = arr.transpose(2, 0, 1, 3).reshape(D, TOK)  # [d, tok]
    out = full.T.astype(np.float32) + np.asarray(bo, np.float32)[None, :]
    return out.reshape(B, L, D)


def run(trace=False, **inputs):
    nc = _get_nc()
    in_maps = _prep_inputs(**inputs)
    res = run_bass_kernel_spmd(nc, in_maps, core_ids=list(range(NCORES)),
                               trace=trace)
    out = _postprocess(res.results, inputs["bo"])
    return out, res


def kernel(**inputs):
    out, _ = run(trace=False, **inputs)
    return out
